# revision 15
# baseline (speedup 1.0000x reference)
"""GraphSAGE-style GNN layer on 8 Trainium2 NeuronCores.

out = relu(W @ concat([features[nodes], mean(features[neigh_idx], 1)], 1).T)

Strategy: data-parallel over the 16384-node batch (2048 nodes/core), feature
table host-cast to bf16. The gather dominates, so it runs through the batched
ext-isa dma_gather (int16 indices, Q7-vectorized descriptor generation)
instead of per-tile indirect DMAs: the host re-buckets the batch's feature
rows into 11 shared 32768-row windows (deduped union across all cores per
2048-row span) so each core fetches 1024 rows per dma_gather op (descriptor-
ring capacity bound) from the op's window. 22 ops land the core's 22528 rows
in one contiguous SBUF buffer laid out [p, (tile slot) f]. Neighbor sum +
feature-major transpose both happen on the PE via identity-matmul PSUM
accumulation (alternating PSUM banks so array fills overlap drains), then a
bf16 matmul with the host-folded weight, ScalarE ReLU, fp32 store [256, 2048].
"""
import numpy as np

N_CORES = 8
NUM_NODES = 1_000_000
F = 256
E = 256
B = 16384
NSAMP = 10
SLOTS = 1 + NSAMP
P = 128
B_LOCAL = B // N_CORES          # 2048
TILES = B_LOCAL // P            # 16
TPG = 2                         # tiles per matmul group
GROUPS = TILES // TPG           # 8
NIDX = B_LOCAL * SLOTS          # 22528 rows per core
NIDX_OP = 1024                  # max rows per dma_gather (ring capacity bound)
OP_SIZES = [256] * 4 + [1024] * 21   # pipeline-ramp op sizes, sum == NIDX
OPS_PER_WIN = 2
N_WIN = NIDX // (OPS_PER_WIN * NIDX_OP)   # 11
WIN = 32768                     # rows per shared table window (int16 reach)
TBL_ROWS = N_WIN * WIN          # 360448

_cache = {}


def _build():
    import concourse.bacc as bacc
    import concourse.mybir as mybir
    import concourse.tile as tile
    from concourse import library_config

    bf16 = mybir.dt.bfloat16
    f32 = mybir.dt.float32

    nc = bacc.Bacc("TRN2", target_bir_lowering=False, debug=False,
                   num_swdge_queues=4)
    feats = nc.dram_tensor("features", [TBL_ROWS, F], bf16, kind="ExternalInput")
    wt = nc.dram_tensor("wt", [2 * F, E], bf16, kind="ExternalInput")
    ident_d = nc.dram_tensor("ident", [P, P], bf16, kind="ExternalInput")
    # per-op int16 window-relative gather indices, 16-partition wrapped and
    # replicated to all 128 partitions (dma_gather Q7 read convention)
    gix = nc.dram_tensor("gix", [P, NIDX // 16], mybir.dt.int16,
                         kind="ExternalInput")
    out = nc.dram_tensor("out", [E, B_LOCAL], f32, kind="ExternalOutput")

    with tile.TileContext(nc) as tc:
        with (
            tc.tile_pool(name="const", bufs=1) as constp,
            tc.tile_pool(name="gather", bufs=1) as gatherp,
            tc.tile_pool(name="combT", bufs=2) as combp,
            tc.tile_pool(name="outs", bufs=3) as outsp,
            tc.tile_pool(name="pstA", bufs=2, space="PSUM") as pstA,
            tc.tile_pool(name="pstB", bufs=2, space="PSUM") as pstB,
            tc.tile_pool(name="psc", bufs=1, space="PSUM") as psc,
            tc.tile_pool(name="psm", bufs=2, space="PSUM") as psm,
        ):
            nc.gpsimd.load_library(library_config.mlp)

            ident = constp.tile([P, P], bf16)
            nc.sync.dma_start(out=ident[:], in_=ident_d.ap())
            # absorb the identity-ready wait on PE (Matmult carries 1 HW wait)
            scratch = psc.tile([P, P], f32, tag="scratch")
            nc.tensor.matmul(out=scratch[:], lhsT=ident[:], rhs=ident[:],
                             start=True, stop=True)

            # weights: wtile[k, c*E+e] = wt[c*128+k, e]
            wtile = constp.tile([P, 4 * E], bf16)
            nc.sync.dma_start(
                out=wtile[:].rearrange("k (c e) -> k c e", c=4),
                in_=wt.ap().rearrange("(c k) e -> k c e", k=P),
            )
            ix = constp.tile([P, NIDX // 16], mybir.dt.int16)
            nc.sync.dma_start(out=ix[:], in_=gix.ap())

            # one contiguous gather buffer: chunk c = tile*SLOTS + slot holds
            # G[p, c*F:(c+1)*F] = feats_row(tile, node p, slot); ops write
            # disjoint chunk ranges so Tile tracks per-slice deps. The first
            # four ops are small (256 rows, one per queue) so SDMA doorbells
            # ring ~6us earlier; the doorbell only fires at emission end.
            G = gatherp.tile([P, NIDX * 2], bf16)
            span = OPS_PER_WIN * NIDX_OP   # positions per window
            start = 0
            for k, sz in enumerate(OP_SIZES):
                w = start // span
                nc.gpsimd.dma_gather(
                    out_ap=G[:, (start // P) * F:((start + sz) // P) * F]
                    .rearrange("p (c f) -> p c f", c=sz // P),
                    in_ap=feats.ap()[w * WIN:(w + 1) * WIN, :],
                    idxs_ap=ix[:, start // 16:(start + sz) // 16],
                    num_idxs=sz,
                    num_idxs_reg=sz,
                    elem_size=F,
                    queue_num=k % 4,
                    single_packet=False,
                )
                start += sz
            assert start == NIDX

            for g in range(GROUPS):
                combT = combp.tile([P, 4 * TPG * P], bf16, tag="combT",
                                   name=f"combT_{g}")
                for bt in range(TPG):
                    base = (g * TPG + bt) * SLOTS * F
                    # transpose self rows + accumulate-transpose neighbor rows.
                    # Chunk 0 of each pair goes to bank A, chunk 1 to bank B,
                    # strictly alternating so PE array fills overlap drains.
                    ptA = pstA.tile([P, 512], f32, tag="ptA")
                    ptB = pstB.tile([P, 512], f32, tag="ptB")
                    nc.tensor.matmul(out=ptA[:, 0:P],
                                     lhsT=G[:, base: base + P],
                                     rhs=ident[:], start=True, stop=True)
                    nc.tensor.matmul(out=ptB[:, 0:P],
                                     lhsT=G[:, base + P: base + 2 * P],
                                     rhs=ident[:], start=True, stop=True)
                    for s in range(1, SLOTS):
                        off = base + s * F
                        nc.tensor.matmul(out=ptA[:, P:2 * P],
                                         lhsT=G[:, off: off + P],
                                         rhs=ident[:],
                                         start=(s == 1), stop=(s == SLOTS - 1))
                        nc.tensor.matmul(out=ptB[:, P:2 * P],
                                         lhsT=G[:, off + P: off + 2 * P],
                                         rhs=ident[:],
                                         start=(s == 1), stop=(s == SLOTS - 1))
                    # bank A holds [self_c0 | nsum_c0] -> combT kc 0 and 2;
                    # bank B holds [self_c1 | nsum_c1] -> combT kc 1 and 3
                    cv = combT[:].rearrange("p (kc n) -> p kc n", kc=4)
                    nc.vector.tensor_copy(
                        out=cv[:, 0::2, bt * P:(bt + 1) * P],
                        in_=ptA[:, 0:2 * P].rearrange("p (c n) -> p c n", c=2))
                    nc.vector.tensor_copy(
                        out=cv[:, 1::2, bt * P:(bt + 1) * P],
                        in_=ptB[:, 0:2 * P].rearrange("p (c n) -> p c n", c=2))
                N = TPG * P
                pm = psm.tile([P, 512], f32, tag="pm")
                for ec in range(2):
                    for kc in range(4):
                        nc.tensor.matmul(
                            out=pm[:, ec * N:(ec + 1) * N],
                            lhsT=wtile[:, kc * E + ec * P: kc * E + (ec + 1) * P],
                            rhs=combT[:, kc * N:(kc + 1) * N],
                            start=(kc == 0), stop=(kc == 3),
                        )
                o = outsp.tile([P, 2 * N], f32, tag="o")
                for ec in range(2):
                    nc.scalar.activation(o[:, ec * N:(ec + 1) * N],
                                         pm[:, ec * N:(ec + 1) * N],
                                         mybir.ActivationFunctionType.Relu)
                nc.sync.dma_start(
                    out=out.ap()[:, g * N:(g + 1) * N]
                    .rearrange("(ec p) n -> p ec n", p=P),
                    in_=o[:].rearrange("p (ec n) -> p ec n", ec=2))
    nc.compile()
    return nc


def _get_nc():
    if "nc" not in _cache:
        _cache["nc"] = _build()
    return _cache["nc"]


def _prep(features, W, nodes, neigh_idx):
    """Host-side: bf16 table re-bucketed into N_WIN shared windows, folded
    bf16 weight, per-core wrapped int16 index lists."""
    import ml_dtypes

    bf16 = ml_dtypes.bfloat16
    feats = np.asarray(features)
    featsb = feats.astype(bf16) if feats.dtype != bf16 else feats
    W = np.asarray(W, dtype=np.float32)
    nodes = np.asarray(nodes).astype(np.int64)
    neigh = np.asarray(neigh_idx).astype(np.int64)

    wt = np.ascontiguousarray(np.concatenate(
        [W[:, :F].T, W[:, F:].T / NSAMP], axis=0).astype(bf16))

    # original row ids in gather-list order: j = (tile*SLOTS + slot)*128 + p
    allgx = np.concatenate([nodes[:, None], neigh], axis=1).reshape(
        N_CORES, TILES, P, SLOTS)
    ids_j = allgx.transpose(0, 1, 3, 2).reshape(N_CORES, NIDX)

    span = OPS_PER_WIN * NIDX_OP  # 2048 positions per window
    table = np.empty((TBL_ROWS, F), dtype=bf16)
    ranks_all = np.empty((N_CORES, NIDX), dtype=np.int16)
    for w in range(N_WIN):
        sub = ids_j[:, w * span:(w + 1) * span]            # [8, 2048]
        uniq = np.unique(sub)
        assert len(uniq) <= WIN, (w, len(uniq))
        table[w * WIN: w * WIN + len(uniq)] = featsb[uniq]
        ranks_all[:, w * span:(w + 1) * span] = np.searchsorted(uniq, sub)
    # wrap each op's indices: position j -> partition j%16, col j//16,
    # replicated to all 128 partitions
    ix16 = np.empty((N_CORES, P, NIDX // 16), dtype=np.int16)
    start = 0
    for sz in OP_SIZES:
        r = ranks_all[:, start:start + sz]
        wrp = r.reshape(N_CORES, sz // 16, 16).transpose(0, 2, 1)
        ix16[:, :, start // 16:(start + sz) // 16] = np.tile(wrp, (1, P // 16, 1))
        start += sz

    ident = np.eye(P, dtype=bf16)
    in_maps = []
    for c in range(N_CORES):
        in_maps.append({"features": table, "wt": wt, "ident": ident,
                        "gix": np.ascontiguousarray(ix16[c])})
    return in_maps


def run(features, W, nodes, neigh_idx, trace=False):
    from concourse.bass_utils import run_bass_kernel_spmd

    in_maps = _prep(features, W, nodes, neigh_idx)
    res = run_bass_kernel_spmd(_get_nc(), in_maps,
                               core_ids=list(range(N_CORES)), trace=trace)
    out = np.concatenate([r["out"] for r in res.results], axis=1)
    return out, res


def kernel(features, W, nodes, neigh_idx):
    out, _ = run(features, W, nodes, neigh_idx)
    return out


# revision 18
# speedup vs baseline: 1.0036x; 1.0036x over previous
"""GraphSAGE-style GNN layer on 8 Trainium2 NeuronCores.

out = relu(W @ concat([features[nodes], mean(features[neigh_idx], 1)], 1).T)

Strategy: data-parallel over the 16384-node batch (2048 nodes/core), feature
table host-cast to bf16. The gather dominates, so it runs through the batched
ext-isa dma_gather (int16 indices, Q7-vectorized descriptor generation)
instead of per-tile indirect DMAs: the host re-buckets the batch's feature
rows into 11 shared 32768-row windows (deduped union across all cores per
2048-row span) so each core fetches 1024 rows per dma_gather op (descriptor-
ring capacity bound) from the op's window. 22 ops land the core's 22528 rows
in one contiguous SBUF buffer laid out [p, (tile slot) f]. Neighbor sum +
feature-major transpose both happen on the PE via identity-matmul PSUM
accumulation (alternating PSUM banks so array fills overlap drains), then a
bf16 matmul with the host-folded weight, ScalarE ReLU, fp32 store [256, 2048].
"""
import numpy as np

N_CORES = 8
NUM_NODES = 1_000_000
F = 256
E = 256
B = 16384
NSAMP = 10
SLOTS = 1 + NSAMP
P = 128
B_LOCAL = B // N_CORES          # 2048
TILES = B_LOCAL // P            # 16
TPG = 2                         # tiles per matmul group
GROUPS = TILES // TPG           # 8
NIDX = B_LOCAL * SLOTS          # 22528 rows per core
NIDX_OP = 1024                  # max rows per dma_gather (ring capacity bound)
OP_SIZES = [1024] * 22          # op sizes, sum == NIDX
OPS_PER_WIN = 2
N_WIN = NIDX // (OPS_PER_WIN * NIDX_OP)   # 11
WIN = 32768                     # rows per shared table window (int16 reach)
TBL_ROWS = N_WIN * WIN          # 360448

_cache = {}


def _build():
    import concourse.bacc as bacc
    import concourse.mybir as mybir
    import concourse.tile as tile
    from concourse import library_config

    bf16 = mybir.dt.bfloat16
    f32 = mybir.dt.float32

    nc = bacc.Bacc("TRN2", target_bir_lowering=False, debug=False,
                   num_swdge_queues=4)
    feats = nc.dram_tensor("features", [TBL_ROWS, F], bf16, kind="ExternalInput")
    wt = nc.dram_tensor("wt", [2 * F, E], bf16, kind="ExternalInput")
    ident_d = nc.dram_tensor("ident", [P, P], bf16, kind="ExternalInput")
    # per-op int16 window-relative gather indices, 16-partition wrapped and
    # replicated to all 128 partitions (dma_gather Q7 read convention)
    gix = nc.dram_tensor("gix", [P, NIDX // 16], mybir.dt.int16,
                         kind="ExternalInput")
    out = nc.dram_tensor("out", [E, B_LOCAL], f32, kind="ExternalOutput")

    with tile.TileContext(nc) as tc:
        with (
            tc.tile_pool(name="const", bufs=1) as constp,
            tc.tile_pool(name="gather", bufs=1) as gatherp,
            tc.tile_pool(name="combT", bufs=2) as combp,
            tc.tile_pool(name="outs", bufs=3) as outsp,
            tc.tile_pool(name="pstA", bufs=2, space="PSUM") as pstA,
            tc.tile_pool(name="pstB", bufs=2, space="PSUM") as pstB,
            tc.tile_pool(name="psc", bufs=1, space="PSUM") as psc,
            tc.tile_pool(name="psm", bufs=2, space="PSUM") as psm,
        ):
            nc.gpsimd.load_library(library_config.mlp)

            ident = constp.tile([P, P], bf16)
            nc.sync.dma_start(out=ident[:], in_=ident_d.ap())
            # absorb the identity-ready wait on PE (Matmult carries 1 HW wait)
            scratch = psc.tile([P, P], f32, tag="scratch")
            nc.tensor.matmul(out=scratch[:], lhsT=ident[:], rhs=ident[:],
                             start=True, stop=True)

            # weights: wtile[k, c*E+e] = wt[c*128+k, e]
            wtile = constp.tile([P, 4 * E], bf16)
            nc.sync.dma_start(
                out=wtile[:].rearrange("k (c e) -> k c e", c=4),
                in_=wt.ap().rearrange("(c k) e -> k c e", k=P),
            )
            ix = constp.tile([P, NIDX // 16], mybir.dt.int16)
            nc.sync.dma_start(out=ix[:], in_=gix.ap())

            # one contiguous gather buffer: chunk c = tile*SLOTS + slot holds
            # G[p, c*F:(c+1)*F] = feats_row(tile, node p, slot); ops write
            # disjoint chunk ranges so Tile tracks per-slice deps.
            G = gatherp.tile([P, NIDX * 2], bf16)
            span = OPS_PER_WIN * NIDX_OP   # positions per window
            start = 0
            for k, sz in enumerate(OP_SIZES):
                w = start // span
                nc.gpsimd.dma_gather(
                    out_ap=G[:, (start // P) * F:((start + sz) // P) * F]
                    .rearrange("p (c f) -> p c f", c=sz // P),
                    in_ap=feats.ap()[w * WIN:(w + 1) * WIN, :],
                    idxs_ap=ix[:, start // 16:(start + sz) // 16],
                    num_idxs=sz,
                    num_idxs_reg=sz,
                    elem_size=F,
                    queue_num=k % 4,
                )
                start += sz
            assert start == NIDX

            for g in range(GROUPS):
                combT = combp.tile([P, 4 * TPG * P], bf16, tag="combT",
                                   name=f"combT_{g}")
                for bt in range(TPG):
                    base = (g * TPG + bt) * SLOTS * F
                    # transpose self rows + accumulate-transpose neighbor rows.
                    # Chunk 0 of each pair goes to bank A, chunk 1 to bank B,
                    # strictly alternating so PE array fills overlap drains.
                    ptA = pstA.tile([P, 512], f32, tag="ptA")
                    ptB = pstB.tile([P, 512], f32, tag="ptB")
                    nc.tensor.matmul(out=ptA[:, 0:P],
                                     lhsT=G[:, base: base + P],
                                     rhs=ident[:], start=True, stop=True)
                    nc.tensor.matmul(out=ptB[:, 0:P],
                                     lhsT=G[:, base + P: base + 2 * P],
                                     rhs=ident[:], start=True, stop=True)
                    for s in range(1, SLOTS):
                        off = base + s * F
                        nc.tensor.matmul(out=ptA[:, P:2 * P],
                                         lhsT=G[:, off: off + P],
                                         rhs=ident[:],
                                         start=(s == 1), stop=(s == SLOTS - 1))
                        nc.tensor.matmul(out=ptB[:, P:2 * P],
                                         lhsT=G[:, off + P: off + 2 * P],
                                         rhs=ident[:],
                                         start=(s == 1), stop=(s == SLOTS - 1))
                    # bank A holds [self_c0 | nsum_c0] -> combT kc 0 and 2;
                    # bank B holds [self_c1 | nsum_c1] -> combT kc 1 and 3
                    cv = combT[:].rearrange("p (kc n) -> p kc n", kc=4)
                    nc.vector.tensor_copy(
                        out=cv[:, 0::2, bt * P:(bt + 1) * P],
                        in_=ptA[:, 0:2 * P].rearrange("p (c n) -> p c n", c=2))
                    nc.vector.tensor_copy(
                        out=cv[:, 1::2, bt * P:(bt + 1) * P],
                        in_=ptB[:, 0:2 * P].rearrange("p (c n) -> p c n", c=2))
                N = TPG * P
                pm = psm.tile([P, 512], f32, tag="pm")
                for ec in range(2):
                    for kc in range(4):
                        nc.tensor.matmul(
                            out=pm[:, ec * N:(ec + 1) * N],
                            lhsT=wtile[:, kc * E + ec * P: kc * E + (ec + 1) * P],
                            rhs=combT[:, kc * N:(kc + 1) * N],
                            start=(kc == 0), stop=(kc == 3),
                        )
                o = outsp.tile([P, 2 * N], f32, tag="o")
                for ec in range(2):
                    nc.scalar.activation(o[:, ec * N:(ec + 1) * N],
                                         pm[:, ec * N:(ec + 1) * N],
                                         mybir.ActivationFunctionType.Relu)
                nc.sync.dma_start(
                    out=out.ap()[:, g * N:(g + 1) * N]
                    .rearrange("(ec p) n -> p ec n", p=P),
                    in_=o[:].rearrange("p (ec n) -> p ec n", ec=2))
    nc.compile()
    return nc


def _get_nc():
    if "nc" not in _cache:
        _cache["nc"] = _build()
    return _cache["nc"]


def _prep(features, W, nodes, neigh_idx):
    """Host-side: bf16 table re-bucketed into N_WIN shared windows, folded
    bf16 weight, per-core wrapped int16 index lists."""
    import ml_dtypes

    bf16 = ml_dtypes.bfloat16
    feats = np.asarray(features)
    featsb = feats.astype(bf16) if feats.dtype != bf16 else feats
    W = np.asarray(W, dtype=np.float32)
    nodes = np.asarray(nodes).astype(np.int64)
    neigh = np.asarray(neigh_idx).astype(np.int64)

    wt = np.ascontiguousarray(np.concatenate(
        [W[:, :F].T, W[:, F:].T / NSAMP], axis=0).astype(bf16))

    # original row ids in gather-list order: j = (tile*SLOTS + slot)*128 + p
    allgx = np.concatenate([nodes[:, None], neigh], axis=1).reshape(
        N_CORES, TILES, P, SLOTS)
    ids_j = allgx.transpose(0, 1, 3, 2).reshape(N_CORES, NIDX)

    span = OPS_PER_WIN * NIDX_OP  # 2048 positions per window
    table = np.empty((TBL_ROWS, F), dtype=bf16)
    ranks_all = np.empty((N_CORES, NIDX), dtype=np.int16)
    for w in range(N_WIN):
        sub = ids_j[:, w * span:(w + 1) * span]            # [8, 2048]
        uniq = np.unique(sub)
        assert len(uniq) <= WIN, (w, len(uniq))
        table[w * WIN: w * WIN + len(uniq)] = featsb[uniq]
        ranks_all[:, w * span:(w + 1) * span] = np.searchsorted(uniq, sub)
    # wrap each op's indices: position j -> partition j%16, col j//16,
    # replicated to all 128 partitions
    ix16 = np.empty((N_CORES, P, NIDX // 16), dtype=np.int16)
    start = 0
    for sz in OP_SIZES:
        r = ranks_all[:, start:start + sz]
        wrp = r.reshape(N_CORES, sz // 16, 16).transpose(0, 2, 1)
        ix16[:, :, start // 16:(start + sz) // 16] = np.tile(wrp, (1, P // 16, 1))
        start += sz

    ident = np.eye(P, dtype=bf16)
    in_maps = []
    for c in range(N_CORES):
        in_maps.append({"features": table, "wt": wt, "ident": ident,
                        "gix": np.ascontiguousarray(ix16[c])})
    return in_maps


def run(features, W, nodes, neigh_idx, trace=False):
    from concourse.bass_utils import run_bass_kernel_spmd

    in_maps = _prep(features, W, nodes, neigh_idx)
    res = run_bass_kernel_spmd(_get_nc(), in_maps,
                               core_ids=list(range(N_CORES)), trace=trace)
    out = np.concatenate([r["out"] for r in res.results], axis=1)
    return out, res


def kernel(features, W, nodes, neigh_idx):
    out, _ = run(features, W, nodes, neigh_idx)
    return out
